# revision 108
# baseline (speedup 1.0000x reference)
"""Distributed Trainium2 kernel for nn_Attention (B=2, N=2048, D=768, H=12).

Sharding: core c = (batch g=c//4, quarter r=c%4) computes the FULL attention
block output for ITS 512 queries: all 12 heads' Q/K/V projections, scores,
softmax, context, and the FC — entirely locally. K/V work is replicated
4x within a batch group, which costs ~25us of extra PE time but removes all
collectives (the previous AllToAll design spent 3x28us on the collective
cores plus a 33us exposed tail waiting on the last exchange).

Key compaction: masked keys (padding_mask==1) are removed on the host; the
device sees only surviving keys padded to a multiple of 128 (n_kc chunks).
Pad slots have zero x_kv rows (k=v=0, exp(0)=1) and a 0 in the ones-column
of V, so they contribute nothing to softmax numerator or denominator.

All matmul operands are bf16; scale 1/8 is folded into wq on the host. PV
runs P-stationary (out [q, d]), so each streamed V column yields 128
context values and the softmax denominator (65th ones-column of V) lands
as a per-partition scalar: normalize = reciprocal [128,4] + tensor_scalar,
then a PE transpose (identity matmul) restores [d, q] for the FC. The 4
query-chunk accumulators share one PSUM bank, so PV zero-fills once and
always accumulates (hardware allows one open accumulation group per bank).
The FC output is produced transposed ([768, 512]) so the bias is a
per-partition scalar; the host transposes back. Emission interleaves
projection/FC units into the per-head attention loops as PE filler so the
activation engine (exp, ~57us) never stalls the PE (~88us busy).
"""

import sys
import numpy as np

sys.path.insert(0, "/opt/trn_rl_repo")

import ml_dtypes

B, N, D, H, HD = 2, 2048, 768, 12, 64
P = 128
NCORES = 8
NQ = 512           # queries per core
SCALE = HD ** (-0.5)

_BF16 = ml_dtypes.bfloat16


def _fix_multi_waits(nc):
    """walrus in this container accepts only ONE semaphore wait per
    instruction; hoist extra waits onto EventSemaphore carriers inserted
    immediately before, on the same engine (program order preserved)."""
    import bass_rust

    for b in nc.main_func.blocks:
        insts = b.instructions
        idx = 0
        while idx < len(insts):
            ins = insts[idx]
            si = ins.sync_info
            if si is None or len(si.on_wait) <= 1:
                idx += 1
                continue
            waits = list(si.on_wait)
            excess, keep = waits[:-1], waits[-1:]
            carriers = []
            for k, w in enumerate(excess):
                e = bass_rust.InstEventSemaphore(
                    name=f"{ins.name}_waitsplit_{k}", ins=[], outs=[]
                )
                e.engine = ins.engine
                esi = e.sync_info
                if esi is None:
                    esi = bass_rust.SyncInfo(on_wait=[], on_update=[])
                esi.on_wait = [w]
                e.sync_info = esi
                if ins.debug is not None:
                    e.debug = ins.debug
                carriers.append(e)
            si.on_wait = keep
            ins.sync_info = si
            for k, e in enumerate(carriers):
                insts.insert(idx + k, e)
            idx += len(carriers) + 1


def build_nc(n_kc=9):
    import concourse.bass as bass
    import concourse.mybir as mybir
    import concourse.tile as tile

    BF16, F32 = mybir.dt.bfloat16, mybir.dt.float32
    AF = mybir.ActivationFunctionType
    ALU = mybir.AluOpType

    KCAP = n_kc * P
    npair = (n_kc + 1) // 2

    nc = bass.Bass()
    xt_ext = nc.declare_dram_parameter("xt", [P, 6, NQ], BF16, isOutput=False)
    xkv_ext = nc.declare_dram_parameter("xkv", [P, 6, KCAP], BF16,
                                        isOutput=False)
    # wq/wk are och-major [p, och, cc, 128] so the och-0 slice is one
    # contiguous 1.5KB run per partition (fast first DMA)
    wq_ext = nc.declare_dram_parameter("wq", [P, 6, 6, P], BF16,
                                       isOutput=False)
    wk_ext = nc.declare_dram_parameter("wk", [P, 6, 6, P], BF16,
                                       isOutput=False)
    wv_ext = nc.declare_dram_parameter("wv", [P, 6, D], BF16, isOutput=False)
    wfc_ext = nc.declare_dram_parameter("wfc", [P, 6, D], BF16, isOutput=False)
    bfc_ext = nc.declare_dram_parameter("bfc", [P, 6], F32, isOutput=False)
    ones_ext = nc.declare_dram_parameter("onesm", [P, n_kc, H], BF16,
                                         isOutput=False)
    ident_ext = nc.declare_dram_parameter("ident", [P, P], BF16,
                                          isOutput=False)
    # bf16 store (host upcasts): halves the tail's output-DMA chain; costs
    # ~0.24% extra rounding on the metric (measured), well under the gate
    out_ext = nc.declare_dram_parameter("out", [D, NQ], BF16, isOutput=True)

    with tile.TileContext(nc) as tc:
        with (
            tc.tile_pool(name="persist", bufs=1) as persist,
            tc.tile_pool(name="pTp", bufs=6) as pTp,
            tc.tile_pool(name="ctxnp", bufs=2) as ctxnp,
            tc.tile_pool(name="accp", bufs=6) as accp,
            tc.tile_pool(name="ps", bufs=3, space="PSUM") as ps,
            tc.tile_pool(name="ctxp", bufs=2, space="PSUM") as ctxp,
        ):
            xt = persist.tile([P, 6, NQ], BF16)
            xkv = persist.tile([P, 6, KCAP], BF16)
            wq = persist.tile([P, 6, 6, P], BF16)
            wk = persist.tile([P, 6, 6, P], BF16)
            wv = persist.tile([P, 6, D], BF16)
            wfc = persist.tile([P, 6, D], BF16)
            bfc = persist.tile([P, 6], F32)
            onesm = persist.tile([P, n_kc, H], BF16)
            kT = persist.tile([P, 6, KCAP], BF16)
            qT = persist.tile([P, 6, NQ], BF16)
            vv = persist.tile([P, n_kc, H, HD + 1], BF16)
            fcin = persist.tile([P, 6, NQ], BF16)
            recb = persist.tile([P, 4, 1], F32)
            ident = persist.tile([P, P], BF16)
            ebias = persist.tile([P, 1], F32)

            # load order tuned so head-0 attention can start ASAP:
            # onesm tiny; xkv strip 0 + wk[och0] unblock k_unit(0,0);
            # remaining xkv strips, then q/v inputs, then the rest.
            nstrip = (KCAP + 511) // 512
            w0 = min(512, KCAP)
            nc.sync.dma_start(wk[:, 0, :, :], wk_ext[:, 0, :, :])
            nc.sync.dma_start(xkv[:, 0:3, 0:w0], xkv_ext[:, 0:3, 0:w0])
            nc.sync.dma_start(xkv[:, 3:6, 0:w0], xkv_ext[:, 3:6, 0:w0])
            for s in range(1, min(2, nstrip)):
                s0, w = s * 512, min(512, KCAP - s * 512)
                nc.sync.dma_start(xkv[:, :, s0:s0 + w],
                                  xkv_ext[:, :, s0:s0 + w])
            nc.sync.dma_start(wv[:, :, 0:384], wv_ext[:, :, 0:384])
            nc.sync.dma_start(onesm[:], ones_ext[:])
            nc.sync.dma_start(wq[:, 0, :, :], wq_ext[:, 0, :, :])
            nc.sync.dma_start(xt[:], xt_ext[:])
            nc.sync.dma_start(wv[:, :, 384:D], wv_ext[:, :, 384:D])
            for s in range(2, nstrip):
                s0, w = s * 512, min(512, KCAP - s * 512)
                nc.sync.dma_start(xkv[:, :, s0:s0 + w],
                                  xkv_ext[:, :, s0:s0 + w])
            nc.sync.dma_start(wk[:, 1:6, :, :], wk_ext[:, 1:6, :, :])
            nc.sync.dma_start(wq[:, 1:6, :, :], wq_ext[:, 1:6, :, :])
            nc.sync.dma_start(wfc[:], wfc_ext[:])
            nc.sync.dma_start(bfc[:], bfc_ext[:])
            nc.sync.dma_start(ident[:], ident_ext[:])
            nc.vector.memset(ebias[:], 0.0)

            # ---- projection units (psums from the shared transient ps ring)
            def k_span(och, s0, w1):
                pk = ps.tile([P, 1024], F32, tag="ps", name="pk")
                for b0 in range(0, w1, 512):
                    bw = min(512, w1 - b0)
                    for cc in range(6):
                        nc.tensor.matmul(
                            pk[:, b0:b0 + bw],
                            lhsT=wk[:, och, cc, :],
                            rhs=xkv[:, cc, s0 + b0:s0 + b0 + bw],
                            start=(cc == 0), stop=(cc == 5),
                        )
                nc.vector.tensor_copy(kT[:, och, s0:s0 + w1], pk[:, 0:w1])

            def k_unit(och, part):
                s0 = 0 if part == 0 else 1024
                k_span(och, s0, min(1024, KCAP - s0))

            def q_unit(och):
                pq = ps.tile([P, 1024], F32, tag="ps", name="pq")
                for cc in range(6):
                    nc.tensor.matmul(
                        pq[:, 0:NQ],
                        lhsT=wq[:, och, cc, :],
                        rhs=xt[:, cc, :],
                        start=(cc == 0), stop=(cc == 5),
                    )
                nc.vector.tensor_copy(qT[:, och, :], pq[:, 0:NQ])

            def v_half(kc, hh):
                # V projection for v-dim half hh (heads 6*hh..6*hh+5): the
                # early key-chunks only need half 0 before head 0 starts,
                # so they can run as soon as the first half of wv lands
                d0 = hh * 384
                pv = ps.tile([P, 1024], F32, tag="ps", name="pv")
                for cc in range(6):
                    nc.tensor.matmul(
                        pv[:, 0:384],
                        lhsT=xkv[:, cc, kc * P:(kc + 1) * P],
                        rhs=wv[:, cc, d0:d0 + 384],
                        start=(cc == 0), stop=(cc == 5),
                    )
                nc.vector.tensor_copy(
                    vv[:, kc, 6 * hh:6 * hh + 6, 0:HD],
                    pv[:, 0:384].rearrange("p (h d) -> p h d", d=HD),
                )
                if hh == 0:
                    nc.gpsimd.tensor_copy(vv[:, kc, :, HD], onesm[:, kc, :])

            def v_unit(kc):
                pv = ps.tile([P, 1024], F32, tag="ps", name="pv")
                for n0, w in ((0, 512), (512, 256)):
                    for cc in range(6):
                        nc.tensor.matmul(
                            pv[:, n0:n0 + w],
                            lhsT=xkv[:, cc, kc * P:(kc + 1) * P],
                            rhs=wv[:, cc, n0:n0 + w],
                            start=(cc == 0), stop=(cc == 5),
                        )
                nc.vector.tensor_copy(
                    vv[:, kc, :, 0:HD],
                    pv[:, 0:D].rearrange("p (h d) -> p h d", d=HD),
                )
                nc.gpsimd.tensor_copy(vv[:, kc, :, HD], onesm[:, kc, :])

            # ---- attention for head h: steps of up to 2 key-chunks; the
            # 3-deep psum ring hides the exp latency behind 2 steps of PE.
            # PV is P-stationary (out [q, d+1]), so each streamed V column
            # yields 128 context values and the softmax denominators land as
            # per-partition scalars.
            def att_head(h, inserts=None):
                och, half = h // 2, h % 2
                rows = slice(half * HD, (half + 1) * HD)
                pctx = ctxp.tile([P, 4, HD + 1], F32, tag="pctx", name="pctx")
                # 4 query-chunk accumulators share one PSUM bank; hardware
                # allows only ONE open accumulation group per bank, so
                # zero-fill once and run every PV matmul in accumulate mode.
                nc.vector.memset(pctx[:], 0.0)

                def kcs_of(t):
                    return range(2 * t, min(2 * t + 2, n_kc))

                def qk(t):
                    pss = ps.tile([P, 1024], F32, tag="ps", name="pss")
                    for j, kc in enumerate(kcs_of(t)):
                        nc.tensor.matmul(
                            pss[:, j * 512:j * 512 + 512],
                            lhsT=kT[rows, och, kc * P:(kc + 1) * P],
                            rhs=qT[rows, och, :],
                            start=True, stop=True,
                        )
                    return pss

                pss_cur = qk(0)
                for t in range(npair):
                    w = 512 * len(kcs_of(t))
                    for fn in (inserts or {}).get(t, []):
                        fn()
                    pT = pTp.tile([P, 1024], BF16, tag="pT")
                    nc.scalar.activation(pT[:, 0:w], pss_cur[:, 0:w], AF.Exp,
                                         bias=ebias[:])
                    if t + 1 < npair:
                        pss_cur = qk(t + 1)
                    for j, kc in enumerate(kcs_of(t)):
                        for qc in range(4):
                            nc.tensor.matmul(
                                pctx[:, qc, :],
                                lhsT=pT[:, j * 512 + qc * P:
                                        j * 512 + (qc + 1) * P],
                                rhs=vv[:, kc, h, :],
                                start=False, stop=(kc == n_kc - 1),
                                skip_group_check=True,
                            )
                return pctx

            # normalization: denominators are per-partition scalars now; the
            # reciprocal (DVE) is emitted right after the head's last PV,
            # the scale + transpose + store are deferred into the NEXT
            # head's stream so the PE never waits on the DVE chain.
            def norm_recip(pctx):
                with nc.allow_low_precision(reason="bf16 softmax recip"):
                    nc.vector.reciprocal(recb[:, :, 0], pctx[:, :, HD])

            def norm_mult(pctx):
                ctxn = ctxnp.tile([P, 4, HD], BF16, tag="ctxn", name="ctxn")
                nc.vector.tensor_tensor(
                    ctxn[:], pctx[:, :, 0:HD],
                    recb[:].broadcast_to((P, 4, HD)), ALU.mult)
                return ctxn

            def norm_fin(h, ctxn):
                half = h % 2
                tps = ps.tile([P, 1024], BF16, tag="ps", name="tps")
                for qc in range(4):
                    nc.tensor.transpose(tps[0:HD, qc * P:(qc + 1) * P],
                                        ctxn[:, qc, :], ident[:])
                nc.vector.tensor_copy(
                    fcin[half * HD:(half + 1) * HD, h // 2, :],
                    tps[0:HD, 0:NQ],
                )

            # FC split in three stages of two cc-chunks each; stages 1a/1b
            # run as PE filler under late heads (their fcin chunks are ready
            # once heads 0-3 / 4-7 are normalized), stage 2 is the tail.
            facc = persist.tile([P, 6, NQ], F32)

            # FC in two stages: stage 0 accumulates cc0-3 (+bias) as chunky
            # PE filler once heads 0-7 are normalized; stage 1 (cc4+cc5,
            # tail-exposed) adds the rest and stores.
            FC_CCS = {0: (0, 1, 2, 3), 1: (4, 5)}

            def fc_stage(oc, stage):
                pft = ps.tile([P, 1024], F32, tag="ps", name="pf")
                pf = pft[:, 0:NQ]
                ccs = FC_CCS[stage]
                for i, cc in enumerate(ccs):
                    nc.tensor.matmul(
                        pf,
                        lhsT=wfc[:, cc, oc * P:(oc + 1) * P],
                        rhs=fcin[:, cc, :],
                        start=(i == 0), stop=(i == len(ccs) - 1),
                    )
                if stage == 0:
                    nc.vector.tensor_scalar(facc[:, oc, :], pf,
                                            bfc[:, oc:oc + 1], None, ALU.add)
                else:
                    acc = accp.tile([P, NQ], BF16, tag="acc", name="acc")
                    nc.vector.tensor_tensor(acc[:], facc[:, oc, :], pf,
                                            ALU.add)
                    nc.sync.dma_start(out_ext[oc * P:(oc + 1) * P, :], acc[:])

            # ---- emission (och-0 K in 512-wide spans so the first matmuls
            # start as soon as the first xkv strip lands; the last span is
            # deferred into head 0 since its kc are consumed last). v2 goes
            # last: its copy latency hides under head 0's first QK, while
            # q0's copy hides under v2's matmuls.
            for s0 in range(0, min(KCAP, 1024), 512):
                k_span(0, s0, min(512, KCAP - s0))
            for kc in range(min(2, n_kc)):
                v_half(kc, 0)
            q_unit(0)
            if n_kc >= 3:
                v_half(2, 0)

            def K(o, part):
                return lambda: k_unit(o, part)

            def Q(o):
                return lambda: q_unit(o)

            def V(k):
                return lambda: v_unit(k)

            def VB(k):
                return lambda: v_half(k, 1)

            def FC(o, st):
                return lambda: fc_stage(o, st)

            # Per-head, per-step filler placement. Constraints: v(kc) before
            # the PV consuming it; kT/qT chunk j before head 2j; FC stage s
            # only after the mult producing its last fcin chunk (emitted at
            # step t=2 of the following head). DVE-heavy units (fc adds)
            # capped per head so the DVE never paces a head.
            k0_tail = ([lambda: k_span(0, 1024, KCAP - 1024)]
                       if KCAP > 1024 else [])
            fill = {
                0: {1: k0_tail + [V(3), V(4)], 2: [V(5), V(6)],
                    3: [V(7), V(8)]},
                1: {1: [K(1, 0), VB(0)], 2: [K(1, 1)], 3: [Q(1)]},
                2: {1: [K(2, 0), VB(1)], 2: [K(2, 1)], 3: [VB(2)]},
                3: {1: [Q(2)], 2: [K(3, 0)]},
                4: {1: [K(3, 1)], 2: [Q(3)]},
                5: {1: [K(4, 0)], 2: [K(4, 1)]},
                6: {1: [Q(4)]},
                7: {1: [K(5, 0)], 2: [K(5, 1)]},
                8: {1: [Q(5)], 4: [FC(0, 0)]},
                9: {1: [FC(1, 0)], 4: [FC(2, 0)]},
                10: {1: [FC(3, 0)], 4: [FC(4, 0)]},
                11: {1: [FC(5, 0)]},
            }
            fc_tail = []
            fc_tail_stages = (1,)
            if n_kc != 9:
                # fallback: generic spread of v/k/q units only; all FC
                # stages run in the tail (correct for any n_kc, slower)
                units = (k0_tail + [VB(k) for k in range(min(3, n_kc))]
                         + [V(k) for k in range(3, n_kc)])
                for och in range(1, 6):
                    units += [K(och, 0)]
                    if KCAP > 1024:
                        units.append(K(och, 1))
                    units.append(Q(och))
                per = -(-len(units) // (H * max(1, npair - 1)))
                fill, ui = {}, 0
                for h in range(H):
                    fill[h] = {}
                    for t in range(1, npair):
                        if ui < len(units):
                            fill[h][t] = units[ui:ui + per]
                            ui += per
                for fn in units[ui:]:
                    fn()
                fc_tail = []
                fc_tail_stages = (0, 1)

            prev = None
            for h in range(H):
                ins = {t: list(us) for t, us in fill[h].items()}
                if prev is not None:
                    tn = min(3, npair - 1)
                    ins.setdefault(tn, [])
                    ins[tn] = [prev] + ins[tn]
                pctx = att_head(h, inserts=ins)
                norm_recip(pctx)
                prev = (lambda hh, pc: lambda: norm_fin(hh, norm_mult(pc)))(
                    h, pctx)
            prev()

            for fn in fc_tail:
                fn()
            for st in fc_tail_stages:
                for oc in range(6):
                    fc_stage(oc, st)

    _fix_multi_waits(nc)
    return nc


def _t6(a):
    """[768, X] channel-major -> [128, 6, X]."""
    x = a.shape[1]
    return np.ascontiguousarray(a.reshape(6, P, x).transpose(1, 0, 2))


def _n_kc_for(padding_mask):
    counts = [(np.asarray(padding_mask[g]) == 0).sum() for g in range(B)]
    return max(1, int(-(-max(counts) // P)))


def _prep_in_maps(inputs, padding_mask, w_qkv, w_fc, b_fc, n_kc=None):
    if n_kc is None:
        n_kc = _n_kc_for(padding_mask)
    KCAP = n_kc * P
    x = np.asarray(inputs, dtype=np.float32)
    pm = np.asarray(padding_mask)
    w_qkv = np.asarray(w_qkv, dtype=np.float32)
    w_fc = np.asarray(w_fc, dtype=np.float32)
    b_fc = np.asarray(b_fc, dtype=np.float32)

    def _t66(a):
        """[768 in, 768 out] -> [128p, 6 och, 6 cc, 128] (och-major)."""
        t = a.reshape(6, P, 6, P)           # (cc, p, och, o)
        return np.ascontiguousarray(t.transpose(1, 2, 0, 3))

    wq_t = _t66((w_qkv[0:D] * SCALE).T).astype(_BF16)
    wk_t = _t66(w_qkv[D:2 * D].T).astype(_BF16)
    wv_t = _t6(w_qkv[2 * D:3 * D].T).astype(_BF16)
    wfc_t = _t6(w_fc.T).astype(_BF16)
    bfc_t = np.ascontiguousarray(b_fc.reshape(6, P).T).astype(np.float32)

    in_maps = []
    for g in range(B):
        idx = np.where(pm[g] == 0)[0]
        cnt = len(idx)
        assert cnt <= KCAP, (cnt, KCAP)
        xkv_rows = np.zeros((KCAP, D), dtype=np.float32)
        xkv_rows[:cnt] = x[g][idx]
        xkv_t = _t6(xkv_rows.T).astype(_BF16)
        flat = np.zeros(KCAP, dtype=np.float32)
        flat[:cnt] = 1.0
        ones = np.zeros((P, n_kc, H), dtype=np.float32)
        ones[:, :, :] = flat.reshape(n_kc, P).T[:, :, None]
        ones = ones.astype(_BF16)
        for r in range(4):
            xt_t = _t6(
                np.ascontiguousarray(x[g][r * NQ:(r + 1) * NQ].T)
            ).astype(_BF16)
            in_maps.append({
                "xt": xt_t,
                "xkv": xkv_t,
                "wq": wq_t,
                "wk": wk_t,
                "wv": wv_t,
                "wfc": wfc_t,
                "bfc": bfc_t,
                "onesm": ones,
                "ident": np.eye(P, dtype=_BF16),
            })
    return in_maps


_CACHED_NC = {}


def get_nc(n_kc=9):
    if n_kc not in _CACHED_NC:
        _CACHED_NC[n_kc] = build_nc(n_kc)
    return _CACHED_NC[n_kc]


def kernel(inputs, padding_mask, w_qkv, w_fc, b_fc):
    from concourse.bass_utils import run_bass_kernel_spmd

    n_kc = _n_kc_for(padding_mask)
    nc = get_nc(n_kc)
    in_maps = _prep_in_maps(inputs, padding_mask, w_qkv, w_fc, b_fc, n_kc)
    res = run_bass_kernel_spmd(nc, in_maps, list(range(NCORES)))
    out = np.empty((B, N, D), dtype=np.float32)
    for c in range(NCORES):
        out[c // 4, (c % 4) * NQ:(c % 4 + 1) * NQ, :] = (
            res.results[c]["out"].astype(np.float32).T)
    return out


# revision 109
# speedup vs baseline: 1.0016x; 1.0016x over previous
"""Distributed Trainium2 kernel for nn_Attention (B=2, N=2048, D=768, H=12).

Sharding: core c = (batch g=c//4, quarter r=c%4) computes the FULL attention
block output for ITS 512 queries: all 12 heads' Q/K/V projections, scores,
softmax, context, and the FC — entirely locally. K/V work is replicated
4x within a batch group, which costs ~25us of extra PE time but removes all
collectives (the previous AllToAll design spent 3x28us on the collective
cores plus a 33us exposed tail waiting on the last exchange).

Key compaction: masked keys (padding_mask==1) are removed on the host; the
device sees only surviving keys padded to a multiple of 128 (n_kc chunks).
Pad slots have zero x_kv rows (k=v=0, exp(0)=1) and a 0 in the ones-column
of V, so they contribute nothing to softmax numerator or denominator.

All matmul operands are bf16; scale 1/8 is folded into wq on the host. PV
runs P-stationary (out [q, d]), so each streamed V column yields 128
context values and the softmax denominator (65th ones-column of V) lands
as a per-partition scalar: normalize = reciprocal [128,4] + tensor_scalar,
then a PE transpose (identity matmul) restores [d, q] for the FC. The 4
query-chunk accumulators share one PSUM bank, so PV zero-fills once and
always accumulates (hardware allows one open accumulation group per bank).
The FC output is produced transposed ([768, 512]) so the bias is a
per-partition scalar; the host transposes back. Emission interleaves
projection/FC units into the per-head attention loops as PE filler so the
activation engine (exp, ~57us) never stalls the PE (~88us busy).
"""

import sys
import numpy as np

sys.path.insert(0, "/opt/trn_rl_repo")

import ml_dtypes

B, N, D, H, HD = 2, 2048, 768, 12, 64
P = 128
NCORES = 8
NQ = 512           # queries per core
SCALE = HD ** (-0.5)

_BF16 = ml_dtypes.bfloat16


def _fix_multi_waits(nc):
    """walrus in this container accepts only ONE semaphore wait per
    instruction; hoist extra waits onto EventSemaphore carriers inserted
    immediately before, on the same engine (program order preserved)."""
    import bass_rust

    for b in nc.main_func.blocks:
        insts = b.instructions
        idx = 0
        while idx < len(insts):
            ins = insts[idx]
            si = ins.sync_info
            if si is None or len(si.on_wait) <= 1:
                idx += 1
                continue
            waits = list(si.on_wait)
            excess, keep = waits[:-1], waits[-1:]
            carriers = []
            for k, w in enumerate(excess):
                e = bass_rust.InstEventSemaphore(
                    name=f"{ins.name}_waitsplit_{k}", ins=[], outs=[]
                )
                e.engine = ins.engine
                esi = e.sync_info
                if esi is None:
                    esi = bass_rust.SyncInfo(on_wait=[], on_update=[])
                esi.on_wait = [w]
                e.sync_info = esi
                if ins.debug is not None:
                    e.debug = ins.debug
                carriers.append(e)
            si.on_wait = keep
            ins.sync_info = si
            for k, e in enumerate(carriers):
                insts.insert(idx + k, e)
            idx += len(carriers) + 1


def build_nc(n_kc=9):
    import concourse.bass as bass
    import concourse.mybir as mybir
    import concourse.tile as tile

    BF16, F32 = mybir.dt.bfloat16, mybir.dt.float32
    AF = mybir.ActivationFunctionType
    ALU = mybir.AluOpType

    KCAP = n_kc * P
    npair = (n_kc + 1) // 2

    nc = bass.Bass()
    xt_ext = nc.declare_dram_parameter("xt", [P, 6, NQ], BF16, isOutput=False)
    xkv_ext = nc.declare_dram_parameter("xkv", [P, 6, KCAP], BF16,
                                        isOutput=False)
    # wq/wk are och-major [p, och, cc, 128] so the och-0 slice is one
    # contiguous 1.5KB run per partition (fast first DMA)
    wq_ext = nc.declare_dram_parameter("wq", [P, 6, 6, P], BF16,
                                       isOutput=False)
    wk_ext = nc.declare_dram_parameter("wk", [P, 6, 6, P], BF16,
                                       isOutput=False)
    wv_ext = nc.declare_dram_parameter("wv", [P, 6, D], BF16, isOutput=False)
    wfc_ext = nc.declare_dram_parameter("wfc", [P, 6, D], BF16, isOutput=False)
    bfc_ext = nc.declare_dram_parameter("bfc", [P, 6], F32, isOutput=False)
    ones_ext = nc.declare_dram_parameter("onesm", [P, n_kc, H], BF16,
                                         isOutput=False)
    ident_ext = nc.declare_dram_parameter("ident", [P, P], BF16,
                                          isOutput=False)
    # bf16 store (host upcasts): halves the tail's output-DMA chain; costs
    # ~0.24% extra rounding on the metric (measured), well under the gate
    out_ext = nc.declare_dram_parameter("out", [D, NQ], BF16, isOutput=True)

    with tile.TileContext(nc) as tc:
        with (
            tc.tile_pool(name="persist", bufs=1) as persist,
            tc.tile_pool(name="pTp", bufs=6) as pTp,
            tc.tile_pool(name="ctxnp", bufs=2) as ctxnp,
            tc.tile_pool(name="accp", bufs=6) as accp,
            tc.tile_pool(name="ps", bufs=3, space="PSUM") as ps,
            tc.tile_pool(name="ctxp", bufs=2, space="PSUM") as ctxp,
        ):
            xt = persist.tile([P, 6, NQ], BF16)
            xkv = persist.tile([P, 6, KCAP], BF16)
            wq = persist.tile([P, 6, 6, P], BF16)
            wk = persist.tile([P, 6, 6, P], BF16)
            wv = persist.tile([P, 6, D], BF16)
            wfc = persist.tile([P, 6, D], BF16)
            bfc = persist.tile([P, 6], F32)
            onesm = persist.tile([P, n_kc, H], BF16)
            kT = persist.tile([P, 6, KCAP], BF16)
            qT = persist.tile([P, 6, NQ], BF16)
            vv = persist.tile([P, n_kc, H, HD + 1], BF16)
            fcin = persist.tile([P, 6, NQ], BF16)
            recb = persist.tile([P, 4, 1], F32)
            ident = persist.tile([P, P], BF16)
            ebias = persist.tile([P, 1], F32)

            # load order tuned so head-0 attention can start ASAP:
            # onesm tiny; xkv strip 0 + wk[och0] unblock k_unit(0,0);
            # remaining xkv strips, then q/v inputs, then the rest.
            nstrip = (KCAP + 511) // 512
            w0 = min(512, KCAP)
            nc.sync.dma_start(wk[:, 0, :, :], wk_ext[:, 0, :, :])
            nc.sync.dma_start(xkv[:, 0:3, 0:w0], xkv_ext[:, 0:3, 0:w0])
            nc.sync.dma_start(xkv[:, 3:6, 0:w0], xkv_ext[:, 3:6, 0:w0])
            for s in range(1, min(2, nstrip)):
                s0, w = s * 512, min(512, KCAP - s * 512)
                nc.sync.dma_start(xkv[:, :, s0:s0 + w],
                                  xkv_ext[:, :, s0:s0 + w])
            nc.sync.dma_start(wv[:, :, 0:384], wv_ext[:, :, 0:384])
            nc.sync.dma_start(onesm[:], ones_ext[:])
            nc.sync.dma_start(wq[:, 0, :, :], wq_ext[:, 0, :, :])
            nc.sync.dma_start(xt[:], xt_ext[:])
            nc.sync.dma_start(wv[:, :, 384:D], wv_ext[:, :, 384:D])
            for s in range(2, nstrip):
                s0, w = s * 512, min(512, KCAP - s * 512)
                nc.sync.dma_start(xkv[:, :, s0:s0 + w],
                                  xkv_ext[:, :, s0:s0 + w])
            nc.sync.dma_start(wk[:, 1:6, :, :], wk_ext[:, 1:6, :, :])
            nc.sync.dma_start(wq[:, 1:6, :, :], wq_ext[:, 1:6, :, :])
            nc.sync.dma_start(wfc[:], wfc_ext[:])
            nc.sync.dma_start(bfc[:], bfc_ext[:])
            nc.sync.dma_start(ident[:], ident_ext[:])
            nc.vector.memset(ebias[:], 0.0)

            # ---- projection units (psums from the shared transient ps ring)
            def k_span(och, s0, w1):
                pk = ps.tile([P, 1024], F32, tag="ps", name="pk")
                for b0 in range(0, w1, 512):
                    bw = min(512, w1 - b0)
                    for cc in range(6):
                        nc.tensor.matmul(
                            pk[:, b0:b0 + bw],
                            lhsT=wk[:, och, cc, :],
                            rhs=xkv[:, cc, s0 + b0:s0 + b0 + bw],
                            start=(cc == 0), stop=(cc == 5),
                        )
                nc.vector.tensor_copy(kT[:, och, s0:s0 + w1], pk[:, 0:w1])

            def k_unit(och, part):
                s0 = 0 if part == 0 else 1024
                k_span(och, s0, min(1024, KCAP - s0))

            def q_unit(och):
                pq = ps.tile([P, 1024], F32, tag="ps", name="pq")
                for cc in range(6):
                    nc.tensor.matmul(
                        pq[:, 0:NQ],
                        lhsT=wq[:, och, cc, :],
                        rhs=xt[:, cc, :],
                        start=(cc == 0), stop=(cc == 5),
                    )
                nc.vector.tensor_copy(qT[:, och, :], pq[:, 0:NQ])

            def v_half(kc, hh):
                # V projection for v-dim half hh (heads 6*hh..6*hh+5): the
                # early key-chunks only need half 0 before head 0 starts,
                # so they can run as soon as the first half of wv lands
                d0 = hh * 384
                pv = ps.tile([P, 1024], F32, tag="ps", name="pv")
                for cc in range(6):
                    nc.tensor.matmul(
                        pv[:, 0:384],
                        lhsT=xkv[:, cc, kc * P:(kc + 1) * P],
                        rhs=wv[:, cc, d0:d0 + 384],
                        start=(cc == 0), stop=(cc == 5),
                    )
                nc.vector.tensor_copy(
                    vv[:, kc, 6 * hh:6 * hh + 6, 0:HD],
                    pv[:, 0:384].rearrange("p (h d) -> p h d", d=HD),
                )
                if hh == 0:
                    nc.gpsimd.tensor_copy(vv[:, kc, :, HD], onesm[:, kc, :])

            def v_unit(kc):
                pv = ps.tile([P, 1024], F32, tag="ps", name="pv")
                for n0, w in ((0, 512), (512, 256)):
                    for cc in range(6):
                        nc.tensor.matmul(
                            pv[:, n0:n0 + w],
                            lhsT=xkv[:, cc, kc * P:(kc + 1) * P],
                            rhs=wv[:, cc, n0:n0 + w],
                            start=(cc == 0), stop=(cc == 5),
                        )
                nc.vector.tensor_copy(
                    vv[:, kc, :, 0:HD],
                    pv[:, 0:D].rearrange("p (h d) -> p h d", d=HD),
                )
                nc.gpsimd.tensor_copy(vv[:, kc, :, HD], onesm[:, kc, :])

            # ---- attention for head h: steps of up to 2 key-chunks; the
            # 3-deep psum ring hides the exp latency behind 2 steps of PE.
            # PV is P-stationary (out [q, d+1]), so each streamed V column
            # yields 128 context values and the softmax denominators land as
            # per-partition scalars.
            def att_head(h, inserts=None):
                och, half = h // 2, h % 2
                rows = slice(half * HD, (half + 1) * HD)
                pctx = ctxp.tile([P, 4, HD + 1], F32, tag="pctx", name="pctx")
                # 4 query-chunk accumulators share one PSUM bank; hardware
                # allows only ONE open accumulation group per bank, so
                # zero-fill once and run every PV matmul in accumulate mode.
                nc.vector.memset(pctx[:], 0.0)

                def kcs_of(t):
                    return range(2 * t, min(2 * t + 2, n_kc))

                def qk(t):
                    pss = ps.tile([P, 1024], F32, tag="ps", name="pss")
                    for j, kc in enumerate(kcs_of(t)):
                        nc.tensor.matmul(
                            pss[:, j * 512:j * 512 + 512],
                            lhsT=kT[rows, och, kc * P:(kc + 1) * P],
                            rhs=qT[rows, och, :],
                            start=True, stop=True,
                        )
                    return pss

                pss_cur = qk(0)
                for t in range(npair):
                    w = 512 * len(kcs_of(t))
                    for fn in (inserts or {}).get(t, []):
                        fn()
                    pT = pTp.tile([P, 1024], BF16, tag="pT")
                    nc.scalar.activation(pT[:, 0:w], pss_cur[:, 0:w], AF.Exp,
                                         bias=ebias[:])
                    if t + 1 < npair:
                        pss_cur = qk(t + 1)
                    for j, kc in enumerate(kcs_of(t)):
                        for qc in range(4):
                            nc.tensor.matmul(
                                pctx[:, qc, :],
                                lhsT=pT[:, j * 512 + qc * P:
                                        j * 512 + (qc + 1) * P],
                                rhs=vv[:, kc, h, :],
                                start=False, stop=(kc == n_kc - 1),
                                skip_group_check=True,
                            )
                return pctx

            # normalization: denominators are per-partition scalars now; the
            # reciprocal (DVE) is emitted right after the head's last PV,
            # the scale + transpose + store are deferred into the NEXT
            # head's stream so the PE never waits on the DVE chain.
            def norm_recip(pctx):
                with nc.allow_low_precision(reason="bf16 softmax recip"):
                    nc.vector.reciprocal(recb[:, :, 0], pctx[:, :, HD])

            def norm_mult(pctx):
                ctxn = ctxnp.tile([P, 4, HD], BF16, tag="ctxn", name="ctxn")
                nc.vector.tensor_tensor(
                    ctxn[:], pctx[:, :, 0:HD],
                    recb[:].broadcast_to((P, 4, HD)), ALU.mult)
                return ctxn

            def norm_fin(h, ctxn):
                half = h % 2
                tps = ps.tile([P, 1024], BF16, tag="ps", name="tps")
                for qc in range(4):
                    nc.tensor.transpose(tps[0:HD, qc * P:(qc + 1) * P],
                                        ctxn[:, qc, :], ident[:])
                nc.vector.tensor_copy(
                    fcin[half * HD:(half + 1) * HD, h // 2, :],
                    tps[0:HD, 0:NQ],
                )

            # FC split in three stages of two cc-chunks each; stages 1a/1b
            # run as PE filler under late heads (their fcin chunks are ready
            # once heads 0-3 / 4-7 are normalized), stage 2 is the tail.
            facc = persist.tile([P, 6, NQ], F32)

            # FC in two stages: stage 0 accumulates cc0-3 (+bias) as chunky
            # PE filler once heads 0-7 are normalized; stage 1 (cc4+cc5,
            # tail-exposed) adds the rest and stores.
            FC_CCS = {0: (0, 1, 2, 3), 1: (4, 5)}

            def fc_stage(oc, stage):
                pft = ps.tile([P, 1024], F32, tag="ps", name="pf")
                pf = pft[:, 0:NQ]
                ccs = FC_CCS[stage]
                for i, cc in enumerate(ccs):
                    nc.tensor.matmul(
                        pf,
                        lhsT=wfc[:, cc, oc * P:(oc + 1) * P],
                        rhs=fcin[:, cc, :],
                        start=(i == 0), stop=(i == len(ccs) - 1),
                    )
                if stage == 0:
                    nc.vector.tensor_scalar(facc[:, oc, :], pf,
                                            bfc[:, oc:oc + 1], None, ALU.add)
                else:
                    acc = accp.tile([P, NQ], BF16, tag="acc", name="acc")
                    nc.vector.tensor_tensor(acc[:], facc[:, oc, :], pf,
                                            ALU.add)
                    nc.sync.dma_start(out_ext[oc * P:(oc + 1) * P, :], acc[:])

            # ---- emission (och-0 K in 512-wide spans so the first matmuls
            # start as soon as the first xkv strip lands; the last span is
            # deferred into head 0 since its kc are consumed last). v2 goes
            # last: its copy latency hides under head 0's first QK, while
            # q0's copy hides under v2's matmuls.
            for s0 in range(0, min(KCAP, 1024), 512):
                k_span(0, s0, min(512, KCAP - s0))
            for kc in range(min(2, n_kc)):
                v_half(kc, 0)
            q_unit(0)
            if n_kc >= 3:
                v_half(2, 0)

            def K(o, part):
                return lambda: k_unit(o, part)

            def Q(o):
                return lambda: q_unit(o)

            def V(k):
                return lambda: v_unit(k)

            def VB(k):
                return lambda: v_half(k, 1)

            def FC(o, st):
                return lambda: fc_stage(o, st)

            # Per-head, per-step filler placement. Constraints: v(kc) before
            # the PV consuming it; kT/qT chunk j before head 2j; FC stage s
            # only after the mult producing its last fcin chunk (emitted at
            # step t=2 of the following head). DVE-heavy units (fc adds)
            # capped per head so the DVE never paces a head.
            k0_tail = ([lambda: k_span(0, 1024, KCAP - 1024)]
                       if KCAP > 1024 else [])
            fill = {
                0: {1: k0_tail + [V(3), V(4)], 2: [V(5), V(6)],
                    3: [V(7), V(8)]},
                1: {1: [K(1, 0), VB(0)], 2: [K(1, 1)], 3: [Q(1)]},
                2: {1: [K(2, 0), VB(1)], 2: [K(2, 1)], 3: [VB(2)]},
                3: {1: [Q(2)], 2: [K(3, 0)]},
                4: {1: [K(3, 1)], 2: [Q(3)]},
                5: {1: [K(4, 0)], 2: [K(4, 1)]},
                6: {1: [Q(4)]},
                7: {1: [K(5, 0)], 2: [K(5, 1)]},
                8: {1: [Q(5)], 4: [FC(0, 0)]},
                9: {1: [FC(1, 0)], 4: [FC(2, 0)]},
                10: {1: [FC(3, 0)], 4: [FC(4, 0)]},
                11: {1: [FC(5, 0)]},
            }
            fc_tail = []
            fc_tail_stages = (1,)
            if n_kc != 9:
                # fallback: generic spread of v/k/q units only; all FC
                # stages run in the tail (correct for any n_kc, slower)
                units = (k0_tail + [VB(k) for k in range(min(3, n_kc))]
                         + [V(k) for k in range(3, n_kc)])
                for och in range(1, 6):
                    units += [K(och, 0)]
                    if KCAP > 1024:
                        units.append(K(och, 1))
                    units.append(Q(och))
                per = -(-len(units) // (H * max(1, npair - 1)))
                fill, ui = {}, 0
                for h in range(H):
                    fill[h] = {}
                    for t in range(1, npair):
                        if ui < len(units):
                            fill[h][t] = units[ui:ui + per]
                            ui += per
                for fn in units[ui:]:
                    fn()
                fc_tail = []
                fc_tail_stages = (0, 1)

            prev = None
            for h in range(H):
                ins = {t: list(us) for t, us in fill[h].items()}
                if prev is not None:
                    tn = min(3, npair - 1)
                    ins.setdefault(tn, [])
                    ins[tn] = ins[tn] + [prev]
                pctx = att_head(h, inserts=ins)
                norm_recip(pctx)
                prev = (lambda hh, pc: lambda: norm_fin(hh, norm_mult(pc)))(
                    h, pctx)
            prev()

            for fn in fc_tail:
                fn()
            for st in fc_tail_stages:
                for oc in range(6):
                    fc_stage(oc, st)

    _fix_multi_waits(nc)
    return nc


def _t6(a):
    """[768, X] channel-major -> [128, 6, X]."""
    x = a.shape[1]
    return np.ascontiguousarray(a.reshape(6, P, x).transpose(1, 0, 2))


def _n_kc_for(padding_mask):
    counts = [(np.asarray(padding_mask[g]) == 0).sum() for g in range(B)]
    return max(1, int(-(-max(counts) // P)))


def _prep_in_maps(inputs, padding_mask, w_qkv, w_fc, b_fc, n_kc=None):
    if n_kc is None:
        n_kc = _n_kc_for(padding_mask)
    KCAP = n_kc * P
    x = np.asarray(inputs, dtype=np.float32)
    pm = np.asarray(padding_mask)
    w_qkv = np.asarray(w_qkv, dtype=np.float32)
    w_fc = np.asarray(w_fc, dtype=np.float32)
    b_fc = np.asarray(b_fc, dtype=np.float32)

    def _t66(a):
        """[768 in, 768 out] -> [128p, 6 och, 6 cc, 128] (och-major)."""
        t = a.reshape(6, P, 6, P)           # (cc, p, och, o)
        return np.ascontiguousarray(t.transpose(1, 2, 0, 3))

    wq_t = _t66((w_qkv[0:D] * SCALE).T).astype(_BF16)
    wk_t = _t66(w_qkv[D:2 * D].T).astype(_BF16)
    wv_t = _t6(w_qkv[2 * D:3 * D].T).astype(_BF16)
    wfc_t = _t6(w_fc.T).astype(_BF16)
    bfc_t = np.ascontiguousarray(b_fc.reshape(6, P).T).astype(np.float32)

    in_maps = []
    for g in range(B):
        idx = np.where(pm[g] == 0)[0]
        cnt = len(idx)
        assert cnt <= KCAP, (cnt, KCAP)
        xkv_rows = np.zeros((KCAP, D), dtype=np.float32)
        xkv_rows[:cnt] = x[g][idx]
        xkv_t = _t6(xkv_rows.T).astype(_BF16)
        flat = np.zeros(KCAP, dtype=np.float32)
        flat[:cnt] = 1.0
        ones = np.zeros((P, n_kc, H), dtype=np.float32)
        ones[:, :, :] = flat.reshape(n_kc, P).T[:, :, None]
        ones = ones.astype(_BF16)
        for r in range(4):
            xt_t = _t6(
                np.ascontiguousarray(x[g][r * NQ:(r + 1) * NQ].T)
            ).astype(_BF16)
            in_maps.append({
                "xt": xt_t,
                "xkv": xkv_t,
                "wq": wq_t,
                "wk": wk_t,
                "wv": wv_t,
                "wfc": wfc_t,
                "bfc": bfc_t,
                "onesm": ones,
                "ident": np.eye(P, dtype=_BF16),
            })
    return in_maps


_CACHED_NC = {}


def get_nc(n_kc=9):
    if n_kc not in _CACHED_NC:
        _CACHED_NC[n_kc] = build_nc(n_kc)
    return _CACHED_NC[n_kc]


def kernel(inputs, padding_mask, w_qkv, w_fc, b_fc):
    from concourse.bass_utils import run_bass_kernel_spmd

    n_kc = _n_kc_for(padding_mask)
    nc = get_nc(n_kc)
    in_maps = _prep_in_maps(inputs, padding_mask, w_qkv, w_fc, b_fc, n_kc)
    res = run_bass_kernel_spmd(nc, in_maps, list(range(NCORES)))
    out = np.empty((B, N, D), dtype=np.float32)
    for c in range(NCORES):
        out[c // 4, (c % 4) * NQ:(c % 4 + 1) * NQ, :] = (
            res.results[c]["out"].astype(np.float32).T)
    return out
